# revision 38
# baseline (speedup 1.0000x reference)
"""Trainium2 Bass kernel for nn_CustomLayer_35682588295215.

Math (from the reference):
    W = scatter_add(zeros(4096, 4096), (row_ids, col_idx), values)
    out[b, s, o] = sum_h x[b, s, h] * W[o, h]          # [4, 2048, 4096]

i.e. a dense [8192, 4096] x [4096, 4096]^T GEMM after densifying the
compressed sparse weight.  The scatter is cheap O(nnz) host-side
preprocessing (np.bincount); the 275-GFLOP GEMM runs on 8 NeuronCores.

Sharding: data-parallel over batch*seq (8192 -> 1024 rows per core), the
densified weight replicated.  Per core:
    out_shard[m, n] = sum_k xT[k, m] * Wt[k, n]
with xT = x_shard^T and Wt = W^T, both laid out host-side so every DMA is
contiguous per partition.

Numerics: the bulk of the contraction runs in bfloat16 (1 PE row/cycle,
2.3e-3 rel err measured) and N_FP8 k256-pairs run in fp8_e4m3 DoubleRow
perf mode (2 rows/cycle).  Error adds in quadrature across K, so a few
fp8 pairs buy PE time at a small, measured accuracy cost (3.75e-2 for
all-fp8, scaled by sqrt(fraction); gate is 2e-2).  Output is stored
bf16 and upcast host-side (negligible extra error, half the out-DMA
drain).

Kernel loop (per core): k-outer / m-inner with all 8 PSUM banks holding
the 8 M-tiles of one 512-wide N-block, so each weight element is read
from HBM exactly once.  PE busy measures within ~1.3% of the matmul
streaming floor (bf16 512-row matmul = 213 ns at 2.4 GHz, fp8 DoubleRow
half that); the remaining wall time is the fixed SPMD preamble (~6 us),
first-tile DMA latency (~5 us), and the post-matmul evict+drain tail
(~8 us).
"""

import sys

for _p in ("/opt/trn_rl_repo",):
    if _p not in sys.path:
        sys.path.insert(0, _p)

import numpy as np
import ml_dtypes

import concourse.bass as bass
import concourse.mybir as mybir
from concourse import bacc, tile
from concourse.bass import ts
from concourse.bass_utils import run_bass_kernel_spmd

N_ROWS = 4096  # output dim (o)
N_COLS = 4096  # input dim (h) = contraction K
B, S = 4, 2048
M_TOT = B * S  # 8192
N_CORES = 8
M = M_TOT // N_CORES  # 1024 rows of x per core

P = 128  # partitions
NB = 512  # N free-dim per PSUM bank
K_TILES = N_COLS // P  # 32
M_TILES = M // P  # 8
N_BLOCKS = N_ROWS // NB  # 8
NSLOTS = K_TILES // 2  # 16 k256 slots per n-block

MM_DT = mybir.dt.bfloat16
F8_DT = mybir.dt.float8e4
NP_BF16 = ml_dtypes.bfloat16
NP_F8 = ml_dtypes.float8_e4m3

# fp8 k256 slots are 1..N_FP8 (slot 0 stays bf16 so the full-width
# start=True matmul zeroes the whole PSUM bank; slot 15 stays bf16 so the
# full-width stop=True covers both halves).  N_FP8=3 measures 1.647e-2
# rel err on the real inputs (gate 2e-2); N_FP8=4 would be 1.892e-2 —
# too close to the gate to be worth 3% more speed.
N_FP8_DEFAULT = 3

# Filled by run(): max-across-traced-cores HW exec time in ns (None if no trace).
LAST_EXEC_NS = None

_CACHED_NC = {}


def _build(
    n_fp8,
    bf16_out=True,
    split0=True,
    scalar_dma=True,
    tail_v2=False,
    head_v2=False,
    warmup=0,
):
    n_bf = K_TILES - 2 * n_fp8  # bf16 k-tiles, packed in slot order
    out_dt = MM_DT if bf16_out else mybir.dt.float32
    nc = bacc.Bacc(None, target_bir_lowering=False, debug=False, num_swdge_queues=3)
    # xs: bf16 x k-tiles [p, kb, m]: xs[p, kb, m] = x_shard[m, K(kb)*128+p]
    xs_d = nc.dram_tensor("xs", [P, n_bf, M], MM_DT, kind="ExternalInput")
    # wt: bf16 W^T k-tiles [p, n, kb, j]: wt[p, n, kb, j] = W[n*512+j, K(kb)*128+p]
    wt_d = nc.dram_tensor("wt", [P, N_BLOCKS, n_bf, NB], MM_DT, kind="ExternalInput")
    if n_fp8:
        x8_d = nc.dram_tensor("x8", [P, n_fp8, 2, M], F8_DT, kind="ExternalInput")
        w8_d = nc.dram_tensor(
            "w8", [P, N_BLOCKS, n_fp8, 2, NB], F8_DT, kind="ExternalInput"
        )
    # Output stored bf16 (host upcasts): halves the out-DMA drain after the
    # last matmul; rounding adds ~8e-4 rel err in quadrature — negligible.
    out_d = nc.dram_tensor("out", [M, N_ROWS], out_dt, kind="ExternalOutput")

    # Slot plan for one n-block: each slot covers 2 k128-tiles.
    # ('bf', bf_pos): two bf16 matmul rounds; ('f8', t): one DoubleRow pair.
    slots = []
    bf_pos = 0
    for s in range(NSLOTS):
        if 1 <= s <= n_fp8:
            slots.append(("f8", s - 1))
        else:
            slots.append(("bf", bf_pos))
            bf_pos += 2

    # x-cache DMA plan: one tile per slot.  Slot 0's k0 tile is split into
    # two 128-KiB m-halves so the first matmul starts after the smallest
    # possible transfer.  Tile for slot s is issued in slot max(0, s-2)'s
    # DMA phase; tiles for the last 4 slots ride the sync queue so block
    # 0's ~12 MiB splits across both queue paths.
    xtiles = []  # (kind, pos, cnt, slot) — cnt==-1/-2: m-half singles of k0
    for s, (kind, pos) in enumerate(slots):
        if s == 0:
            if split0:
                xtiles.append(("bf", 0, -1, s))
                xtiles.append(("bf", 0, -2, s))
            else:
                xtiles.append(("bf", 0, 1, s))
            xtiles.append(("bf", 1, 1, s))
        elif kind == "bf":
            xtiles.append(("bf", pos, 2, s))
        else:
            xtiles.append(("f8", pos, 2, s))
    xs_sched = {s: [] for s in range(NSLOTS)}
    for ti, (kind, pos, cnt, s) in enumerate(xtiles):
        xs_sched[max(0, s - 2)].append(ti)
    sync_slots = set(range(NSLOTS - 4, NSLOTS))

    with tile.TileContext(nc) as tc:
        with (
            tc.tile_pool(name="xs1_pool", bufs=3) as xs1_pool,
            tc.tile_pool(name="xs_pool", bufs=max(1, NSLOTS - 1 - n_fp8)) as xs_pool,
            tc.tile_pool(name="x8_pool", bufs=max(1, n_fp8)) as x8_pool,
            tc.tile_pool(name="wt_pool", bufs=12) as wt_pool,
            tc.tile_pool(name="out_pool", bufs=10) as out_pool,
            tc.tile_pool(name="psum", bufs=8, space="PSUM") as psum_pool,
        ):
            xs_t = [None] * len(xtiles)

            if warmup:
                # Dependency-free matmul chain on zeroed SBUF: burns through
                # the tensor engine's DVFS ramp (0.65/1.2 GHz for its first
                # ~3 us of activity) during the dead time while the first
                # x/weight DMAs are in flight, so the real stream starts at
                # full clock.
                wu_s = wt_pool.tile([P, NB], MM_DT)
                nc.vector.memset(wu_s[:], 0)
                wu_ps = psum_pool.tile([P, NB], mybir.dt.float32, name="ps", tag="ps")
                for i in range(warmup):
                    nc.tensor.matmul(
                        wu_ps[:],
                        wu_s[:, 0:P],
                        wu_s[:],
                        start=(i == 0),
                        stop=(i == warmup - 1),
                    )

            for n in range(N_BLOCKS):
                psums = None
                for s, (kind, pos) in enumerate(slots):
                    if head_v2 and n == 0 and s == 0:
                        # Emit slot-0 x DMAs before the weight DMAs: the
                        # first x half rides the sync queue ahead of the
                        # first weight half, so the first matmul's two
                        # inputs arrive back-to-back on the earliest queue.
                        for ti in xs_sched[0]:
                            xkind, xpos, xcnt, xslot = xtiles[ti]
                            if xkind == "bf" and xcnt < 0:
                                half = -xcnt - 1
                                xt = xs1_pool.tile(
                                    [P, 1, M // 2], MM_DT, name="xs", tag="xs"
                                )
                                src = xs_d[
                                    :,
                                    xpos : xpos + 1,
                                    half * (M // 2) : (half + 1) * (M // 2),
                                ]
                            elif xkind == "bf":
                                pool = xs1_pool if xcnt == 1 else xs_pool
                                xt = pool.tile([P, xcnt, M], MM_DT, name="xs", tag="xs")
                                src = xs_d[:, xpos : xpos + xcnt, :]
                            else:
                                xt = x8_pool.tile([P, 2, M], F8_DT, name="x8", tag="xs")
                                src = x8_d[:, xpos, :, :]
                            xs_eng = nc.sync if ti == 0 else nc.gpsimd
                            xs_eng.dma_start(xt[:], src)
                            xs_t[ti] = xt
                    if kind == "bf":
                        wt_t = wt_pool.tile([P, 2, NB], MM_DT, name="wt", tag="wt")
                        if split0 and s == 0 and n == 0:
                            # k0/k1 weight halves as separate DMAs so the
                            # first matmul waits on only 128 KiB of weights.
                            nc.sync.dma_start(
                                wt_t[:, 0:1, :], wt_d[:, n, pos : pos + 1, :]
                            )
                            nc.sync.dma_start(
                                wt_t[:, 1:2, :], wt_d[:, n, pos + 1 : pos + 2, :]
                            )
                        else:
                            nc.sync.dma_start(wt_t[:], wt_d[:, n, pos : pos + 2, :])
                    else:
                        wt_t = wt_pool.tile([P, 2, NB], F8_DT, name="w8", tag="wt")
                        nc.sync.dma_start(wt_t[:], w8_d[:, n, pos, :, :])
                    if n == 0 and not (head_v2 and s == 0):
                        for ti in xs_sched[s]:
                            xkind, xpos, xcnt, xslot = xtiles[ti]
                            if xkind == "bf" and xcnt < 0:
                                # m-half single of k-tile xpos: [P, 1, M/2]
                                half = -xcnt - 1
                                xt = xs1_pool.tile(
                                    [P, 1, M // 2], MM_DT, name="xs", tag="xs"
                                )
                                src = xs_d[
                                    :, xpos : xpos + 1, half * (M // 2) : (half + 1) * (M // 2)
                                ]
                            elif xkind == "bf":
                                pool = xs1_pool if xcnt == 1 else xs_pool
                                xt = pool.tile([P, xcnt, M], MM_DT, name="xs", tag="xs")
                                src = xs_d[:, xpos : xpos + xcnt, :]
                            else:
                                xt = x8_pool.tile([P, 2, M], F8_DT, name="x8", tag="xs")
                                src = x8_d[:, xpos, :, :]
                            xs_eng = nc.sync if xslot in sync_slots else nc.gpsimd
                            xs_eng.dma_start(xt[:], src)
                            xs_t[ti] = xt
                    if s == 0:
                        psums = [
                            psum_pool.tile(
                                [P, NB], mybir.dt.float32, name="ps", tag="ps"
                            )
                            for _ in range(M_TILES)
                        ]
                    xt_s = [xs_t[ti] for ti in range(len(xtiles)) if xtiles[ti][3] == s]
                    if kind == "bf":
                        for ks in range(2):
                            first = s == 0 and ks == 0
                            last = s == NSLOTS - 1 and ks == 1
                            for m in range(M_TILES):
                                if s == 0 and split0 and ks == 0:
                                    # xt_s = [k0_half0, k0_half1, k1_single]
                                    xop = xt_s[m // 4][:, 0, ts(m % 4, P)]
                                elif s == 0:
                                    xop = xt_s[-1 if ks else 0][:, 0, ts(m, P)]
                                else:
                                    xop = xt_s[0][:, ks, ts(m, P)]
                                nc.tensor.matmul(
                                    psums[m][:],
                                    xop,
                                    wt_t[:, ks, :],
                                    start=first,
                                    stop=last,
                                )
                    else:
                        x8_t = xt_s[0]
                        for m in range(M_TILES):
                            for h in range(2):
                                nc.tensor.matmul(
                                    psums[m][:, ts(h, NB // 2)],
                                    x8_t[:, :, ts(m, P)],
                                    wt_t[:, :, ts(h, NB // 2)],
                                    start=False,
                                    stop=False,
                                    perf_mode=mybir.MatmulPerfMode.DoubleRow,
                                )
                # Evictions split across vector+scalar so the 8 PSUM banks
                # free ~2x sooner at block boundaries (next block's first
                # matmuls wait on bank release).  The last block's out DMAs
                # fan out over 4 queues to shorten the post-matmul drain.
                if tail_v2 and n == N_BLOCKS - 1:
                    # Last block: same copies, but the queue map gives the
                    # last-finishing tile (m=7) the sync queue to itself
                    # (weights are done by now) so its transfer never
                    # queues behind earlier tiles.
                    qmap = [
                        nc.gpsimd, nc.scalar, nc.sync, nc.gpsimd,
                        nc.scalar, nc.gpsimd, nc.scalar, nc.sync,
                    ]
                    for m in range(M_TILES):
                        ot = out_pool.tile([P, NB], out_dt)
                        if m % 2 == 0:
                            nc.vector.tensor_copy(ot[:], psums[m][:])
                        else:
                            nc.scalar.copy(ot[:], psums[m][:])
                        qmap[m].dma_start(out_d[ts(m, P), ts(n, NB)], ot[:])
                    continue
                for m in range(M_TILES):
                    ot = out_pool.tile([P, NB], out_dt)
                    if m % 2 == 0:
                        nc.vector.tensor_copy(ot[:], psums[m][:])
                    else:
                        nc.scalar.copy(ot[:], psums[m][:])
                    if scalar_dma and n == N_BLOCKS - 1:
                        out_eng = (nc.gpsimd, nc.sync, nc.scalar)[m % 3]
                    else:
                        out_eng = nc.gpsimd if m % 2 == 0 else nc.sync
                    out_eng.dma_start(out_d[ts(m, P), ts(n, NB)], ot[:])
    nc.compile()
    return nc


def _get_nc(n_fp8, bf16_out, split0, scalar_dma, tail_v2, head_v2, warmup):
    key = (n_fp8, bf16_out, split0, scalar_dma, tail_v2, head_v2, warmup)
    if key not in _CACHED_NC:
        _CACHED_NC[key] = _build(
            n_fp8, bf16_out, split0, scalar_dma, tail_v2, head_v2, warmup
        )
    return _CACHED_NC[key]


def _densify_wt(values, col_idx, row_ids):
    # Wt[h, o] = sum of values[i] with col_idx[i] == h, row_ids[i] == o
    idx = col_idx.astype(np.int64) * N_ROWS + row_ids.astype(np.int64)
    wt = np.bincount(idx, weights=values.astype(np.float64), minlength=N_COLS * N_ROWS)
    return wt.astype(np.float32).reshape(N_COLS, N_ROWS)


def _install_ntff_hook():
    """The agent image's antenv package lacks axon_hooks; recreate the tiny
    get/set registry and register the ctypes NTFF hook from trn_agent_boot
    so run_bass_kernel_spmd(trace=True) can capture profiles under axon."""
    import types

    if "antenv.axon_hooks" in sys.modules:
        return
    import antenv
    from trn_agent_boot.trn_boot import _ntff_profile_via_ctypes

    mod = types.ModuleType("antenv.axon_hooks")
    mod._hook = _ntff_profile_via_ctypes("/opt/axon/libaxon_pjrt.so")

    def get_axon_ntff_profile_hook():
        return mod._hook

    def set_axon_ntff_profile_hook(h):
        mod._hook = h

    mod.get_axon_ntff_profile_hook = get_axon_ntff_profile_hook
    mod.set_axon_ntff_profile_hook = set_axon_ntff_profile_hook
    sys.modules["antenv.axon_hooks"] = mod
    antenv.axon_hooks = mod


def kernel(
    x,
    values,
    col_idx,
    row_ids,
    trace=False,
    n_fp8=N_FP8_DEFAULT,
    bf16_out=True,
    split0=True,
    scalar_dma=True,
    tail_v2=False,
    head_v2=False,
    warmup=12,
):
    global LAST_EXEC_NS
    if trace:
        _install_ntff_hook()
    x = np.ascontiguousarray(np.asarray(x, dtype=np.float32))
    wt = _densify_wt(np.asarray(values), np.asarray(col_idx), np.asarray(row_ids))

    n_bf = K_TILES - 2 * n_fp8
    bf_k = [0, 1] + list(range(2 + 2 * n_fp8, K_TILES))
    f8_k = list(range(2, 2 + 2 * n_fp8))

    # wr[kb, p, n, j] = Wt[kb*128+p, n*512+j]
    wr = wt.reshape(K_TILES, P, N_BLOCKS, NB)
    wt_l = np.ascontiguousarray(
        wr[bf_k].transpose(1, 2, 0, 3).astype(NP_BF16)
    )  # [P, NBLK, n_bf, NB]
    if n_fp8:
        w8_l = np.ascontiguousarray(
            wr[f8_k]
            .reshape(n_fp8, 2, P, N_BLOCKS, NB)
            .transpose(2, 3, 0, 1, 4)
            .astype(NP_F8)
        )  # [P, NBLK, n_fp8, 2, NB]

    xf = x.reshape(M_TOT, N_COLS)
    in_maps = []
    for c in range(N_CORES):
        xsh = xf[c * M : (c + 1) * M]  # [1024, 4096]
        # kt[kb, p, m] = xsh[m, kb*128+p]
        kt = np.ascontiguousarray(xsh.T).reshape(K_TILES, P, M)
        im = {
            "xs": np.ascontiguousarray(kt[bf_k].transpose(1, 0, 2).astype(NP_BF16)),
            "wt": wt_l,
        }
        if n_fp8:
            im["x8"] = np.ascontiguousarray(
                kt[f8_k].reshape(n_fp8, 2, P, M).transpose(2, 0, 1, 3).astype(NP_F8)
            )
            im["w8"] = w8_l
        in_maps.append(im)

    nc = _get_nc(n_fp8, bf16_out, split0, scalar_dma, tail_v2, head_v2, warmup)
    res = run_bass_kernel_spmd(
        nc, in_maps, core_ids=list(range(N_CORES)), trace=trace
    )
    LAST_EXEC_NS = res.exec_time_ns

    out = np.concatenate([r["out"] for r in res.results], axis=0)
    return out.astype(np.float32).reshape(B, S, N_ROWS)
